# revision 1
# baseline (speedup 1.0000x reference)
"""GATNet forward on 8 TRN2 NeuronCores (Bass/Tile, SPMD).

Math (reference):
    h  = mean_L(x @ lin_w + lin_b)           [N, CIN]
    xt = (h @ gat_w).reshape(N, H, D)
    alpha_e = leaky(att_dst . xt[col] + att_src . xt[row])
    out[t] = sum_e softmax_seg(alpha)[e] * xt[row_e]  (+ gat_bias)

Device algorithm (per core, nodes/targets sharded 8 ways):
  phase 1: x tiles -> mean over L (DVE reduce) -> transpose (PE) ->
           fp32r matmul with folded Wfull = [W2 | W2@As | W2@Ad]/L
           (W2 = lin_w@gat_w, device-computed once) -> per-node row
           [xt(256) | s_src(4) | s_dst(4)] bf16 -> local DRAM table.
  AllGather the node table (bf16) across the 8 cores.
  phase 2: edges pre-sorted by target into 128-target blocks (host),
           lo/hi split for int16 dma_gather; per 128-edge chunk gather
           node rows, w = exp(leaky(s_src+s_dst)), rhs = [w*xt | w],
           matmul against host-built one-hot S accumulating
           [numer | denom] in PSUM; out = numer/denom + bias.
"""
import sys

sys.path.insert(0, "/opt/trn_rl_repo")

import numpy as np
import ml_dtypes

import concourse.bass as bass
import concourse.bacc as bacc
import concourse.mybir as mybir
import concourse.tile as tile
from concourse.masks import make_identity

BF16 = ml_dtypes.bfloat16
FP8 = ml_dtypes.float8_e4m3

f32 = mybir.dt.float32
f32r = mybir.dt.float32r
bf16 = mybir.dt.bfloat16
fp8e4 = mybir.dt.float8e4
i16 = mybir.dt.int16
P = 128


def default_cfg():
    return dict(
        N=50000, L=10, CIN=300, HEADS=4, DOUT=64, E=800000, NEG=0.2,
        NCORES=8, G=2,
    )


def derive_cfg(cfg):
    c = dict(cfg)
    c["OUT"] = c["HEADS"] * c["DOUT"]            # 256
    c["XTW"] = c["OUT"] + 8                      # 264: xt | s_src(4) | s_dst(4)
    c["ROWL"] = ((c["XTW"] * 2 + 255) // 256) * 128  # bf16 row padded to 256B mult
    c["SROW"] = 128                              # s_dst table row (bf16, 256B)
    n_per = -(-c["N"] // c["NCORES"])            # ceil
    c["NP"] = ((n_per + P - 1) // P) * P          # per-core padded nodes
    c["NB"] = c["NP"] // P                        # blocks per core
    c["NPAD"] = c["NP"] * c["NCORES"]
    c["HALF"] = c["NPAD"] // 2
    assert c["HALF"] < 32768 and c["NPAD"] - c["HALF"] < 32768
    c["NTGT"] = c["N"] // c["NCORES"]             # (informational)
    c["NG"] = -(-c["NB"] // c["G"])
    # k-tiling of CIN for the phase-1 matmul
    kt, rem = [], c["CIN"]
    while rem > 0:
        kt.append(min(128, rem))
        rem -= kt[-1]
    c["KT"] = kt
    return c


def _wrap16(idx, width):
    """int16 indices -> [128, width] wrapped-16 layout (pos i -> [i%16, i//16])."""
    n = len(idx)
    assert n % 16 == 0
    out = np.zeros((P, width), np.int16)
    w = np.asarray(idx, np.int16).reshape(n // 16, 16).T  # [16, n/16]
    out[:, : n // 16] = np.tile(w, (8, 1))
    return out


def prep_inputs(cfg, x, edge_index, lin_w, lin_b, gat_w, att, gat_bias):
    """Host-side sharding + index/selector construction. Returns (in_maps, CL, CH)."""
    c = cfg
    N, L, CIN, OUT = c["N"], c["L"], c["CIN"], c["OUT"]
    H, D = c["HEADS"], c["DOUT"]
    NP, NB, NTGT, HALF, G = c["NP"], c["NB"], c["NTGT"], c["HALF"], c["G"]

    x = np.asarray(x, np.float32).reshape(N, L * CIN)
    lin_w = np.asarray(lin_w, np.float32)
    lin_b = np.asarray(lin_b, np.float32)
    gat_w = np.asarray(gat_w, np.float32)
    att = np.asarray(att, np.float32)
    gat_bias = np.asarray(gat_bias, np.float32)

    # attention projection matrices [OUT, 4]
    Ad = np.zeros((OUT, H), np.float32)
    As = np.zeros((OUT, H), np.float32)
    for h in range(H):
        Ad[h * D:(h + 1) * D, h] = att[0, h, 0:D]
        As[h * D:(h + 1) * D, h] = att[0, h, D:2 * D]

    b2 = lin_b @ gat_w                                # [OUT]
    bfull = np.concatenate([b2, b2 @ As, b2 @ Ad]).astype(np.float32)  # [XTW]
    bfull_rep = np.tile(bfull[None, :], (P, 1))
    bias_rep = np.tile(gat_bias[None, :], (P, 1))

    # edges + self loops, grouped by target core/block
    row = np.concatenate([np.asarray(edge_index[0], np.int64), np.arange(N)])
    col = np.concatenate([np.asarray(edge_index[1], np.int64), np.arange(N)])
    core_of = col // NP                               # target owner = node owner
    lt = col - core_of * NP                            # local target id
    blk = lt // P
    tin = lt % P

    # first pass: per (core, block) lo/hi counts -> CL, CH
    key = (core_of * NB + blk).astype(np.int64)
    is_lo = row < HALF
    nlo = np.bincount(key[is_lo], minlength=c["NCORES"] * NB)
    nhi = np.bincount(key[~is_lo], minlength=c["NCORES"] * NB)
    CL = int(-(-nlo.max() // P))
    CH = int(-(-nhi.max() // P))
    CT = CL + CH

    order = np.lexsort((np.where(is_lo, 0, 1), key))   # by block, lo first
    row_s, lt_s, tin_s, key_s, islo_s = (
        row[order], lt[order], tin[order], key[order], is_lo[order])

    in_maps = []
    NG = c["NG"]
    for cid in range(c["NCORES"]):
        # ---- x shard (zero-pad nodes) ----
        n0 = cid * NP
        xs = np.zeros((NP, L * CIN), np.float32)
        hi_n = min(N, n0 + NP)
        if hi_n > n0:
            xs[: hi_n - n0] = x[n0:hi_n]

        ilo = np.zeros((NG, P, G * CL * 8), np.int16)
        ihi = np.zeros((NG, P, G * CH * 8), np.int16)
        icol = np.zeros((NG, P, G * CT * 8), np.int16)
        S = np.zeros((NB, P, CT * P), FP8)

        for b in range(NB):
            k = cid * NB + b
            lo_sel = (key_s == k) & islo_s
            hi_sel = (key_s == k) & ~islo_s
            r_lo, t_lo = row_s[lo_sel], tin_s[lo_sel]
            r_hi, t_hi = row_s[hi_sel], tin_s[hi_sel]

            pos_lo = np.zeros(CL * P, np.int16)
            pos_lo[: len(r_lo)] = r_lo.astype(np.int16)
            pos_hi = np.zeros(CH * P, np.int16)
            pos_hi[: len(r_hi)] = (r_hi - HALF).astype(np.int16)
            pos_col = np.zeros(CT * P, np.int16)
            pos_col[: len(t_lo)] = (b * P + t_lo).astype(np.int16)
            pos_col[CL * P: CL * P + len(t_hi)] = (b * P + t_hi).astype(np.int16)

            # S one-hot: position p (chunk c = p//128) -> target t
            pl = np.arange(len(r_lo))
            S[b, pl % P, (pl // P) * P + t_lo] = 1.0
            ph = CL * P + np.arange(len(r_hi))
            S[b, ph % P, (ph // P) * P + t_hi] = 1.0

            # empty targets (only padded target rows): give them one dummy
            # edge (idx 0 already) so denom > 0 and output stays finite
            present = np.zeros(P, bool)
            present[t_lo] = True
            present[t_hi] = True
            free_pos = len(r_lo)  # first unused lo position
            for t in np.nonzero(~present)[0]:
                assert free_pos < CL * P, "no pad slot for empty target"
                S[b, free_pos % P, (free_pos // P) * P + t] = 1.0
                pos_col[free_pos] = b * P + t
                free_pos += 1

            g, boff = b // G, b % G
            ilo[g, :, boff * CL * 8:(boff + 1) * CL * 8] = _wrap16(pos_lo, CL * 8)
            ihi[g, :, boff * CH * 8:(boff + 1) * CH * 8] = _wrap16(pos_hi, CH * 8)
            icol[g, :, boff * CT * 8:(boff + 1) * CT * 8] = _wrap16(pos_col, CT * 8)

        in_maps.append({
            "x": xs,
            "lin_wT": np.ascontiguousarray(lin_w.T),
            "gat_w": gat_w,
            "gat_wT": np.ascontiguousarray(gat_w.T),
            "Ad": Ad, "As": As,
            "bfull_rep": bfull_rep, "bias_rep": bias_rep,
            "ilo": ilo, "ihi": ihi, "icol": icol, "S": S,
        })
    return in_maps, CL, CH


# ---------------------------------------------------------------------------
# device kernel builder
# ---------------------------------------------------------------------------

def build_nc(cfg, CL, CH, phases="full"):
    c = cfg
    L, CIN, OUT, XTW = c["L"], c["CIN"], c["OUT"], c["XTW"]
    NP, NB, NPAD, HALF = c["NP"], c["NB"], c["NPAD"], c["HALF"]
    ROWL, SROW, G, NG = c["ROWL"], c["SROW"], c["G"], c["NG"]
    KT = c["KT"]
    NK = len(KT)
    CT = CL + CH
    H, D = c["HEADS"], c["DOUT"]
    NEG = c["NEG"]
    # contraction tilings
    co_t = [min(128, OUT - i) for i in range(0, OUT, 128)]   # OUT tiles (co)

    nc = bacc.Bacc(num_devices=c["NCORES"])
    x_ext = nc.declare_dram_parameter("x", [NP, L * CIN], f32, isOutput=False)
    lwT_ext = nc.declare_dram_parameter("lin_wT", [CIN, CIN], f32, isOutput=False)
    gw_ext = nc.declare_dram_parameter("gat_w", [CIN, OUT], f32, isOutput=False)
    gwT_ext = nc.declare_dram_parameter("gat_wT", [OUT, CIN], f32, isOutput=False)
    ad_ext = nc.declare_dram_parameter("Ad", [OUT, H], f32, isOutput=False)
    as_ext = nc.declare_dram_parameter("As", [OUT, H], f32, isOutput=False)
    bf_ext = nc.declare_dram_parameter("bfull_rep", [P, XTW], f32, isOutput=False)
    bias_ext = nc.declare_dram_parameter("bias_rep", [P, OUT], f32, isOutput=False)
    ilo_ext = nc.declare_dram_parameter("ilo", [NG, P, G * CL * 8], i16, isOutput=False)
    ihi_ext = nc.declare_dram_parameter("ihi", [NG, P, G * CH * 8], i16, isOutput=False)
    icol_ext = nc.declare_dram_parameter("icol", [NG, P, G * CT * 8], i16, isOutput=False)
    s_ext = nc.declare_dram_parameter("S", [NB, P, CT * P], fp8e4, isOutput=False)
    out_ext = nc.declare_dram_parameter("out", [NP, OUT], f32, isOutput=True)

    xts_loc = nc.dram_tensor("xts_loc", [NP, ROWL], bf16)
    sdst_loc = nc.dram_tensor("sdst_loc", [NP, SROW], bf16)
    xts_all = nc.dram_tensor("xts_all", [NPAD, ROWL], bf16, addr_space="Shared")

    with tile.TileContext(nc) as tc:
        with tc.tile_pool(name="persist", bufs=1) as pp:
            ident = pp.tile([P, P], f32)
            make_identity(nc, ident[:])
            bfull_t = pp.tile([P, XTW], f32)
            nc.sync.dma_start(out=bfull_t[:], in_=bf_ext[:])
            # Wfull k-tiles [ksz, XTW] f32
            wfull = [pp.tile([KT[k], XTW], f32r, name=f"wfull{k}") for k in range(NK)]

            # ---------------- preamble: fold weights on device ----------------
            with tc.tile_pool(name="pre", bufs=2) as pre, \
                 tc.tile_pool(name="prep", bufs=1, space="PSUM") as prep:
                # loads
                lwT = {}
                k0 = 0
                for k in range(NK):
                    m0 = 0
                    for m in range(NK):
                        t = pre.tile([KT[k], KT[m]], f32, name=f"lwT{k}{m}", bufs=1)
                        nc.sync.dma_start(
                            out=t[:], in_=lwT_ext[k0:k0 + KT[k], m0:m0 + KT[m]])
                        lwT[(k, m)] = t
                        m0 += KT[m]
                    k0 += KT[k]
                gw = []
                k0 = 0
                for k in range(NK):
                    t = pre.tile([KT[k], OUT], f32, name=f"gw{k}", bufs=1)
                    nc.sync.dma_start(out=t[:], in_=gw_ext[k0:k0 + KT[k], :])
                    gw.append(t)
                    k0 += KT[k]
                gwT = {}
                q0 = 0
                for q in range(len(co_t)):
                    m0 = 0
                    for m in range(NK):
                        t = pre.tile([co_t[q], KT[m]], f32, name=f"gwT{q}{m}", bufs=1)
                        nc.sync.dma_start(
                            out=t[:], in_=gwT_ext[q0:q0 + co_t[q], m0:m0 + KT[m]])
                        gwT[(q, m)] = t
                        m0 += KT[m]
                    q0 += co_t[q]
                ad_t, as_t = [], []
                q0 = 0
                for q in range(len(co_t)):
                    t1 = pre.tile([co_t[q], H], f32, name=f"ad{q}", bufs=1)
                    nc.sync.dma_start(out=t1[:], in_=ad_ext[q0:q0 + co_t[q], :])
                    t2 = pre.tile([co_t[q], H], f32, name=f"as{q}", bufs=1)
                    nc.sync.dma_start(out=t2[:], in_=as_ext[q0:q0 + co_t[q], :])
                    ad_t.append(t1)
                    as_t.append(t2)
                    q0 += co_t[q]

                # g_d/g_s [cm, H] = gat_w @ A  (lhsT = gwT tiles)
                gd_sb, gs_sb = [], []
                for m in range(NK):
                    for name, amat, dst in (("gd", ad_t, gd_sb), ("gs", as_t, gs_sb)):
                        ps = prep.tile([KT[m], H], f32, space="PSUM",
                                       name="gps", uniquify=True)
                        for q in range(len(co_t)):
                            nc.tensor.matmul(
                                ps[:], lhsT=gwT[(q, m)][:], rhs=amat[q][:],
                                start=(q == 0), stop=(q == len(co_t) - 1))
                        sb = pre.tile([KT[m], H], f32, name=f"{name}sb{m}", bufs=1)
                        nc.scalar.copy(sb[:], ps[:])
                        dst.append(sb)

                # per output k-tile (ci rows of Wfull)
                for m in range(NK):
                    w2ps = prep.tile([KT[m], OUT], f32, space="PSUM", name="w2ps", uniquify=True)
                    for k in range(NK):
                        nc.tensor.matmul(w2ps[:], lhsT=lwT[(k, m)][:], rhs=gw[k][:],
                                         start=(k == 0), stop=(k == NK - 1))
                    wsps = prep.tile([KT[m], H], f32, space="PSUM", name="wsps", uniquify=True)
                    for k in range(NK):
                        nc.tensor.matmul(wsps[:], lhsT=lwT[(k, m)][:], rhs=gs_sb[k][:],
                                         start=(k == 0), stop=(k == NK - 1))
                    wdps = prep.tile([KT[m], H], f32, space="PSUM", name="wdps", uniquify=True)
                    for k in range(NK):
                        nc.tensor.matmul(wdps[:], lhsT=lwT[(k, m)][:], rhs=gd_sb[k][:],
                                         start=(k == 0), stop=(k == NK - 1))
                    sc = 1.0 / L
                    nc.scalar.mul(wfull[m][:, 0:OUT], w2ps[:], sc)
                    nc.scalar.mul(wfull[m][:, OUT:OUT + H], wsps[:], sc)
                    nc.scalar.mul(wfull[m][:, OUT + H:XTW], wdps[:], sc)

            # ---------------- phase 1 ----------------
            with tc.tile_pool(name="p1", bufs=4) as p1, \
                 tc.tile_pool(name="p1p", bufs=2, space="PSUM") as p1p, \
                 tc.tile_pool(name="p1pt", bufs=3, space="PSUM") as p1pt:
                for b in range(NB):
                    x_t = p1.tile([P, L * CIN], f32, name="x_t")
                    nc.sync.dma_start(out=x_t[:], in_=x_ext[b * P:(b + 1) * P, :])
                    xm = p1.tile([P, CIN], f32, name="xm")
                    nc.vector.tensor_reduce(
                        out=xm[:],
                        in_=x_t[:].rearrange("p (l c) -> p c l", l=L),
                        op=mybir.AluOpType.add, axis=mybir.AxisListType.X)
                    xt_ps = p1p.tile([P, XTW], f32, space="PSUM", name="xt_ps")
                    k0 = 0
                    for k in range(NK):
                        tr_ps = p1pt.tile([KT[k], P], f32, space="PSUM", name="tr_ps")
                        nc.tensor.transpose(
                            tr_ps[:], xm[:, k0:k0 + KT[k]], ident[:])
                        xmT = p1.tile([KT[k], P], f32r, name="xmT")
                        nc.scalar.copy(xmT[:], tr_ps[:])
                        nc.tensor.matmul(
                            xt_ps[:], lhsT=xmT[:], rhs=wfull[k][:],
                            start=(k == 0), stop=(k == NK - 1))
                        k0 += KT[k]
                    xts_sb = p1.tile([P, ROWL], bf16, name="xts_sb")
                    nc.scalar.memzero(xts_sb[:, XTW:ROWL])
                    nc.vector.tensor_add(xts_sb[:, 0:XTW], xt_ps[:], bfull_t[:])
                    nc.scalar.dma_start(
                        out=xts_loc[b * P:(b + 1) * P, :], in_=xts_sb[:])
                    sd_sb = p1.tile([P, SROW], bf16, name="sd_sb")
                    nc.scalar.memzero(sd_sb[:, H:SROW])
                    nc.vector.tensor_copy(sd_sb[:, 0:H], xts_sb[:, OUT + H:XTW])
                    nc.scalar.dma_start(
                        out=sdst_loc[b * P:(b + 1) * P, :], in_=sd_sb[:])

            if phases != "p1":
                nc.gpsimd.collective_compute(
                    "AllGather", mybir.AluOpType.bypass,
                    replica_groups=[list(range(c["NCORES"]))],
                    ins=[xts_loc[:]], outs=[xts_all[:]])

        # ---------------- phase 2 ----------------
        with tc.tile_pool(name="p2", bufs=2) as p2, \
             tc.tile_pool(name="p2s", bufs=2) as p2s, \
             tc.tile_pool(name="p2p", bufs=4, space="PSUM") as p2p:
            bias_t = p2s.tile([P, OUT], f32, bufs=1)
            nc.sync.dma_start(out=bias_t[:], in_=bias_ext[:])
            ng_run = {"full": NG, "g1": 1, "g2": 2}.get(phases, 0)
            for g in range(ng_run):
                b0 = g * G
                nb = min(NB - b0, G)
                ilo_t = p2.tile([P, G * CL * 8], i16, name="ilo_t")
                nc.sync.dma_start(out=ilo_t[:, :nb * CL * 8],
                                  in_=ilo_ext[g, :, :nb * CL * 8])
                ihi_t = p2.tile([P, G * CH * 8], i16, name="ihi_t")
                nc.sync.dma_start(out=ihi_t[:, :nb * CH * 8],
                                  in_=ihi_ext[g, :, :nb * CH * 8])
                icol_t = p2.tile([P, G * CT * 8], i16, name="icol_t")
                nc.sync.dma_start(out=icol_t[:, :nb * CT * 8],
                                  in_=icol_ext[g, :, :nb * CT * 8])
                s_t = p2.tile([P, G * CT * P], fp8e4, name="s_t")
                for j in range(nb):
                    nc.sync.dma_start(
                        out=s_t[:, j * CT * P:(j + 1) * CT * P],
                        in_=s_ext[b0 + j, :, :])

                xg_lo = p2.tile([P, G * CL * ROWL], bf16, name="xg_lo", bufs=3)
                nc.gpsimd.dma_gather(
                    out_ap=xg_lo[:, :nb * CL * ROWL].rearrange(
                        "p (c e) -> p c e", e=ROWL),
                    in_ap=xts_all[0:HALF, :], idxs_ap=ilo_t[:, :nb * CL * 8],
                    num_idxs=nb * CL * P, num_idxs_reg=nb * CL * P,
                    elem_size=ROWL, single_packet=False)
                xg_hi = p2.tile([P, G * CH * ROWL], bf16, name="xg_hi", bufs=3)
                nc.gpsimd.dma_gather(
                    out_ap=xg_hi[:, :nb * CH * ROWL].rearrange(
                        "p (c e) -> p c e", e=ROWL),
                    in_ap=xts_all[HALF:NPAD, :], idxs_ap=ihi_t[:, :nb * CH * 8],
                    num_idxs=nb * CH * P, num_idxs_reg=nb * CH * P,
                    elem_size=ROWL, single_packet=False)
                sd_t = p2.tile([P, G * CT * SROW], bf16, name="sd_t", bufs=3)
                nc.gpsimd.dma_gather(
                    out_ap=sd_t[:, :nb * CT * SROW].rearrange(
                        "p (c e) -> p c e", e=SROW),
                    in_ap=sdst_loc[:], idxs_ap=icol_t[:, :nb * CT * 8],
                    num_idxs=nb * CT * P, num_idxs_reg=nb * CT * P,
                    elem_size=SROW, single_packet=False)

                xl3 = xg_lo[:].rearrange("p (c e) -> p c e", e=ROWL)
                xh3 = xg_hi[:].rearrange("p (c e) -> p c e", e=ROWL)
                sd3 = sd_t[:].rearrange("p (c e) -> p c e", e=SROW)
                s3 = s_t[:].rearrange("p (c t) -> p c t", t=P)

                for j in range(nb):
                    b = b0 + j
                    # logits = s_src(gathered) + s_dst(gathered)  [P, CT*4]
                    lg = p2.tile([P, CT * H], f32, name="lg")
                    lg3 = lg[:].rearrange("p (c s) -> p c s", s=H)
                    nc.vector.tensor_add(
                        lg3[:, 0:CL, :],
                        xl3[:, j * CL:(j + 1) * CL, OUT:OUT + H],
                        sd3[:, j * CT:j * CT + CL, 0:H])
                    nc.vector.tensor_add(
                        lg3[:, CL:CT, :],
                        xh3[:, j * CH:(j + 1) * CH, OUT:OUT + H],
                        sd3[:, j * CT + CL:(j + 1) * CT, 0:H])
                    # leaky relu
                    lgm = p2.tile([P, CT * H], f32, name="lgm")
                    nc.vector.tensor_scalar_mul(lgm[:], lg[:], NEG)
                    nc.vector.tensor_tensor(
                        out=lg[:], in0=lgm[:], in1=lg[:], op=mybir.AluOpType.max)
                    # w = exp
                    w_bf = p2.tile([P, CT * H], bf16, name="w_bf")
                    nc.scalar.activation(
                        w_bf[:], lg[:], mybir.ActivationFunctionType.Exp)
                    # pair-duplicate then int32-broadcast to [P, CT*OUT]
                    wpp = p2.tile([P, CT * H * 2], bf16, name="wpp")
                    wpp3 = wpp[:].rearrange("p (k d) -> p k d", d=2)
                    wb3 = w_bf[:].rearrange("p (k o) -> p k o", o=1)
                    nc.vector.tensor_copy(wpp3[:, :, 0:1], wb3)
                    nc.vector.tensor_copy(wpp3[:, :, 1:2], wb3)
                    wr = p2.tile([P, CT * OUT], bf16, name="wr")
                    wr_i3 = wr[:].bitcast(mybir.dt.int32).rearrange(
                        "p (k r) -> p k r", r=D // 2)
                    wpp_i3 = wpp[:].bitcast(mybir.dt.int32).rearrange(
                        "p (k o) -> p k o", o=1)
                    nc.vector.tensor_copy(
                        wr_i3, wpp_i3.to_broadcast([P, CT * H, D // 2]))
                    # rhs = [w * xt | w]
                    rhs = p2.tile([P, CT * (OUT + H)], bf16, name="rhs")
                    rhs3 = rhs[:].rearrange("p (c e) -> p c e", e=OUT + H)
                    wr3 = wr[:].rearrange("p (c e) -> p c e", e=OUT)
                    nc.vector.tensor_mul(
                        rhs3[:, 0:CL, 0:OUT],
                        xl3[:, j * CL:(j + 1) * CL, 0:OUT], wr3[:, 0:CL, :])
                    nc.vector.tensor_mul(
                        rhs3[:, CL:CT, 0:OUT],
                        xh3[:, j * CH:(j + 1) * CH, 0:OUT], wr3[:, CL:CT, :])
                    nc.vector.tensor_copy(
                        rhs3[:, :, OUT:OUT + H],
                        w_bf[:].rearrange("p (c s) -> p c s", s=H))
                    # accumulate [numer | denom] over chunks
                    ps_b = p2p.tile([P, OUT + H], f32, space="PSUM", name="ps_b")
                    for cc in range(CT):
                        nc.tensor.matmul(
                            ps_b[:], lhsT=s3[:, j * CT + cc, :],
                            rhs=rhs3[:, cc, :],
                            start=(cc == 0), stop=(cc == CT - 1))
                    # finalize: out = numer * (1/denom) + bias
                    rd = p2.tile([P, H], f32, name="rd")
                    nc.vector.reciprocal(rd[:], ps_b[:, OUT:OUT + H])
                    rdr = p2.tile([P, OUT], f32, name="rdr")
                    rd3 = rd[:].rearrange("p (h o) -> p h o", o=1)
                    nc.vector.tensor_copy(
                        rdr[:].rearrange("p (h e) -> p h e", e=D),
                        rd3.to_broadcast([P, H, D]))
                    outv = p2.tile([P, OUT], f32, name="outv")
                    nc.vector.tensor_mul(outv[:], ps_b[:, 0:OUT], rdr[:])
                    nc.vector.tensor_add(outv[:], outv[:], bias_t[:])
                    nc.scalar.dma_start(
                        out=out_ext[b * P:(b + 1) * P, :], in_=outv[:])

    nc.finalize()
    return nc


# ---------------------------------------------------------------------------
# entry points
# ---------------------------------------------------------------------------

def run_spmd(nc, in_maps, cfg, trace=False):
    from concourse.bass_utils import run_bass_kernel_spmd

    return run_bass_kernel_spmd(
        nc, in_maps, list(range(cfg["NCORES"])), trace=trace)


def assemble_output(cfg, results):
    out = np.zeros((cfg["N"], cfg["OUT"]), np.float32)
    for cid in range(cfg["NCORES"]):
        n0 = cid * cfg["NP"]
        n1 = min(cfg["N"], n0 + cfg["NP"])
        if n1 > n0:
            out[n0:n1] = results[cid]["out"][0:n1 - n0]
    return out


def run_full(inputs, trace=False):
    cfg = derive_cfg(default_cfg())
    in_maps, CL, CH = prep_inputs(
        cfg, inputs["x"], inputs["edge_index"], inputs["lin_w"],
        inputs["lin_b"], inputs["gat_w"], inputs["att"], inputs["gat_bias"])
    nc = build_nc(cfg, CL, CH)
    r = run_spmd(nc, in_maps, cfg, trace=trace)
    return assemble_output(cfg, r.results), r


def kernel(**inputs):
    out, _ = run_full(inputs, trace=False)
    return out



# revision 17
# speedup vs baseline: 3.5360x; 3.5360x over previous
"""GATNet forward on 8 TRN2 NeuronCores (Bass/Tile, SPMD).

Math (reference):
    h  = mean_L(x @ lin_w + lin_b)           [N, CIN]
    xt = (h @ gat_w).reshape(N, H, D)
    alpha_e = leaky(att_dst . xt[col] + att_src . xt[row])
    out[t] = sum_e softmax_seg(alpha)[e] * xt[row_e]  (+ gat_bias)

Device algorithm (per core, nodes/targets sharded 8 ways):
  phase 1: x tiles -> mean over L (DVE reduce) -> transpose (PE) ->
           fp32r matmul with folded Wfull = [W2 | W2@As | W2@Ad]/L
           (W2 = lin_w@gat_w, device-computed once) -> per-node row
           [xt(256) | s_src(4) | s_dst(4)] bf16 -> local DRAM table.
  AllGather the node table (bf16) across the 8 cores.
  phase 2: edges pre-sorted by target into 128-target blocks (host),
           lo/hi split for int16 dma_gather; per 128-edge chunk gather
           node rows, w = exp(leaky(s_src+s_dst)), rhs = [w*xt | w],
           matmul against host-built one-hot S accumulating
           [numer | denom] in PSUM; out = numer/denom + bias.
"""
import sys

sys.path.insert(0, "/opt/trn_rl_repo")

import numpy as np
import ml_dtypes

import concourse.bass as bass
import concourse.bacc as bacc
import concourse.mybir as mybir
import concourse.tile as tile
from concourse.masks import make_identity

BF16 = ml_dtypes.bfloat16
FP8 = ml_dtypes.float8_e4m3

f32 = mybir.dt.float32
f32r = mybir.dt.float32r
bf16 = mybir.dt.bfloat16
fp8e4 = mybir.dt.float8e4
i16 = mybir.dt.int16
P = 128


def default_cfg():
    return dict(
        N=50000, L=10, CIN=300, HEADS=4, DOUT=64, E=800000, NEG=0.2,
        NCORES=8, G=2,
    )


def derive_cfg(cfg):
    c = dict(cfg)
    c["OUT"] = c["HEADS"] * c["DOUT"]            # 256
    c["XTW"] = c["OUT"] + 8                      # 264: xt | s_src(4) | s_dst(4)
    c["ROWL"] = ((c["XTW"] * 2 + 255) // 256) * 128  # bf16 row padded to 256B mult
    c["SROW"] = 128                              # s_dst table row (bf16, 256B)
    n_per = -(-c["N"] // c["NCORES"])            # ceil
    c["NP"] = ((n_per + P - 1) // P) * P          # per-core padded nodes
    c["NB"] = c["NP"] // P                        # blocks per core
    c["NPAD"] = c["NP"] * c["NCORES"]
    c["HALF"] = c["NPAD"] // 2
    assert c["HALF"] < 32768 and c["NPAD"] - c["HALF"] < 32768
    c["NTGT"] = c["N"] // c["NCORES"]             # (informational)
    c["NG"] = -(-c["NB"] // c["G"])
    # k-tiling of CIN for the phase-1 matmul
    kt, rem = [], c["CIN"]
    while rem > 0:
        kt.append(min(128, rem))
        rem -= kt[-1]
    c["KT"] = kt
    return c


def _wrap16(idx, width):
    """int16 indices -> [128, width] wrapped-16 layout (pos i -> [i%16, i//16])."""
    n = len(idx)
    assert n % 16 == 0
    out = np.zeros((P, width), np.int16)
    w = np.asarray(idx, np.int16).reshape(n // 16, 16).T  # [16, n/16]
    out[:, : n // 16] = np.tile(w, (8, 1))
    return out


def prep_inputs(cfg, x, edge_index, lin_w, lin_b, gat_w, att, gat_bias):
    """Host-side sharding + index/selector construction. Returns (in_maps, CL, CH)."""
    c = cfg
    N, L, CIN, OUT = c["N"], c["L"], c["CIN"], c["OUT"]
    H, D = c["HEADS"], c["DOUT"]
    NP, NB, NTGT, HALF, G = c["NP"], c["NB"], c["NTGT"], c["HALF"], c["G"]

    x = np.asarray(x, np.float32).reshape(N, L * CIN)
    lin_w = np.asarray(lin_w, np.float32)
    lin_b = np.asarray(lin_b, np.float32)
    gat_w = np.asarray(gat_w, np.float32)
    att = np.asarray(att, np.float32)
    gat_bias = np.asarray(gat_bias, np.float32)

    # attention projection matrices [OUT, 4]
    Ad = np.zeros((OUT, H), np.float32)
    As = np.zeros((OUT, H), np.float32)
    for h in range(H):
        Ad[h * D:(h + 1) * D, h] = att[0, h, 0:D]
        As[h * D:(h + 1) * D, h] = att[0, h, D:2 * D]

    b2 = lin_b @ gat_w                                # [OUT]
    bfull = np.concatenate([b2, b2 @ As, b2 @ Ad]).astype(np.float32)  # [XTW]
    bfull_rep = np.tile(bfull[None, :], (P, 1))
    bias_rep = np.tile(gat_bias[None, :], (P, 1))

    # edges + self loops, grouped by target core/block
    row = np.concatenate([np.asarray(edge_index[0], np.int64), np.arange(N)])
    col = np.concatenate([np.asarray(edge_index[1], np.int64), np.arange(N)])
    core_of = col // NP                               # target owner = node owner
    lt = col - core_of * NP                            # local target id
    blk = lt // P
    tin = lt % P

    # first pass: per (core, block) lo/hi counts -> CL, CH
    key = (core_of * NB + blk).astype(np.int64)
    is_lo = row < HALF
    nlo = np.bincount(key[is_lo], minlength=c["NCORES"] * NB)
    nhi = np.bincount(key[~is_lo], minlength=c["NCORES"] * NB)
    CL = int(-(-nlo.max() // P))
    CH = int(-(-nhi.max() // P))
    CT = CL + CH

    order = np.lexsort((np.where(is_lo, 0, 1), key))   # by block, lo first
    row_s, lt_s, tin_s, key_s, islo_s = (
        row[order], lt[order], tin[order], key[order], is_lo[order])

    in_maps = []
    NG = c["NG"]
    for cid in range(c["NCORES"]):
        # ---- x shard (zero-pad nodes) ----
        n0 = cid * NP
        xs = np.zeros((NP, L * CIN), np.float32)
        hi_n = min(N, n0 + NP)
        if hi_n > n0:
            xs[: hi_n - n0] = x[n0:hi_n]

        ilo = np.zeros((NG, P, G * CL * 8), np.int16)
        ihi = np.zeros((NG, P, G * CH * 8), np.int16)
        icol = np.zeros((NG, P, G * CT * 8), np.int16)
        S = np.zeros((NB, P, CT * P), FP8)

        for b in range(NB):
            k = cid * NB + b
            lo_sel = (key_s == k) & islo_s
            hi_sel = (key_s == k) & ~islo_s
            r_lo, t_lo = row_s[lo_sel], tin_s[lo_sel]
            r_hi, t_hi = row_s[hi_sel], tin_s[hi_sel]

            pos_lo = np.zeros(CL * P, np.int16)
            pos_lo[: len(r_lo)] = r_lo.astype(np.int16)
            pos_hi = np.zeros(CH * P, np.int16)
            pos_hi[: len(r_hi)] = (r_hi - HALF).astype(np.int16)
            pos_col = np.zeros(CT * P, np.int16)
            pos_col[: len(t_lo)] = (b * P + t_lo).astype(np.int16)
            pos_col[CL * P: CL * P + len(t_hi)] = (b * P + t_hi).astype(np.int16)

            # S one-hot: position p (chunk c = p//128) -> target t
            pl = np.arange(len(r_lo))
            S[b, pl % P, (pl // P) * P + t_lo] = 1.0
            ph = CL * P + np.arange(len(r_hi))
            S[b, ph % P, (ph // P) * P + t_hi] = 1.0

            # empty targets (only padded target rows): give them one dummy
            # edge (idx 0 already) so denom > 0 and output stays finite
            present = np.zeros(P, bool)
            present[t_lo] = True
            present[t_hi] = True
            free_pos = len(r_lo)  # first unused lo position
            for t in np.nonzero(~present)[0]:
                assert free_pos < CL * P, "no pad slot for empty target"
                S[b, free_pos % P, (free_pos // P) * P + t] = 1.0
                pos_col[free_pos] = b * P + t
                free_pos += 1

            g, boff = b // G, b % G
            ilo[g, :, boff * CL * 8:(boff + 1) * CL * 8] = _wrap16(pos_lo, CL * 8)
            ihi[g, :, boff * CH * 8:(boff + 1) * CH * 8] = _wrap16(pos_hi, CH * 8)
            icol[g, :, boff * CT * 8:(boff + 1) * CT * 8] = _wrap16(pos_col, CT * 8)

        in_maps.append({
            "x": xs,
            "lin_wT": np.ascontiguousarray(lin_w.T),
            "gat_w": gat_w,
            "gat_wT": np.ascontiguousarray(gat_w.T),
            "Ad": Ad, "As": As,
            "bfull_rep": bfull_rep, "bias_rep": bias_rep,
            "ilo": ilo, "ihi": ihi, "icol": icol, "S": S,
        })
    return in_maps, CL, CH


# ---------------------------------------------------------------------------
# device kernel builder
# ---------------------------------------------------------------------------

def build_nc(cfg, CL, CH, phases="full"):
    c = cfg
    L, CIN, OUT, XTW = c["L"], c["CIN"], c["OUT"], c["XTW"]
    NP, NB, NPAD, HALF = c["NP"], c["NB"], c["NPAD"], c["HALF"]
    ROWL, SROW, G, NG = c["ROWL"], c["SROW"], c["G"], c["NG"]
    KT = c["KT"]
    NK = len(KT)
    CT = CL + CH
    H, D = c["HEADS"], c["DOUT"]
    NEG = c["NEG"]
    # contraction tilings
    co_t = [min(128, OUT - i) for i in range(0, OUT, 128)]   # OUT tiles (co)

    nc = bacc.Bacc(num_devices=c["NCORES"])
    x_ext = nc.declare_dram_parameter("x", [NP, L * CIN], f32, isOutput=False)
    lwT_ext = nc.declare_dram_parameter("lin_wT", [CIN, CIN], f32, isOutput=False)
    gw_ext = nc.declare_dram_parameter("gat_w", [CIN, OUT], f32, isOutput=False)
    gwT_ext = nc.declare_dram_parameter("gat_wT", [OUT, CIN], f32, isOutput=False)
    ad_ext = nc.declare_dram_parameter("Ad", [OUT, H], f32, isOutput=False)
    as_ext = nc.declare_dram_parameter("As", [OUT, H], f32, isOutput=False)
    bf_ext = nc.declare_dram_parameter("bfull_rep", [P, XTW], f32, isOutput=False)
    bias_ext = nc.declare_dram_parameter("bias_rep", [P, OUT], f32, isOutput=False)
    ilo_ext = nc.declare_dram_parameter("ilo", [NG, P, G * CL * 8], i16, isOutput=False)
    ihi_ext = nc.declare_dram_parameter("ihi", [NG, P, G * CH * 8], i16, isOutput=False)
    icol_ext = nc.declare_dram_parameter("icol", [NG, P, G * CT * 8], i16, isOutput=False)
    s_ext = nc.declare_dram_parameter("S", [NB, P, CT * P], fp8e4, isOutput=False)
    out_ext = nc.declare_dram_parameter("out", [NP, OUT], f32, isOutput=True)

    xts_loc = nc.dram_tensor("xts_loc", [NP, ROWL], bf16)
    sdst_loc = nc.dram_tensor("sdst_loc", [NP, SROW], bf16)
    xts_all = nc.dram_tensor("xts_all", [NPAD, ROWL], bf16, addr_space="Shared")

    with tile.TileContext(nc) as tc:
        with tc.tile_pool(name="persist", bufs=1) as pp:
            ident = pp.tile([P, P], f32)
            make_identity(nc, ident[:])
            bfull_t = pp.tile([P, XTW], f32)
            nc.sync.dma_start(out=bfull_t[:], in_=bf_ext[:])
            # Wfull k-tiles [ksz, XTW] f32
            wfull = [pp.tile([KT[k], XTW], f32r, name=f"wfull{k}") for k in range(NK)]

            # ---------------- preamble: fold weights on device ----------------
            with tc.tile_pool(name="pre", bufs=2) as pre, \
                 tc.tile_pool(name="prep", bufs=1, space="PSUM") as prep:
                # loads
                lwT = {}
                k0 = 0
                for k in range(NK):
                    m0 = 0
                    for m in range(NK):
                        t = pre.tile([KT[k], KT[m]], f32, name=f"lwT{k}{m}", bufs=1)
                        nc.sync.dma_start(
                            out=t[:], in_=lwT_ext[k0:k0 + KT[k], m0:m0 + KT[m]])
                        lwT[(k, m)] = t
                        m0 += KT[m]
                    k0 += KT[k]
                gw = []
                k0 = 0
                for k in range(NK):
                    t = pre.tile([KT[k], OUT], f32, name=f"gw{k}", bufs=1)
                    nc.sync.dma_start(out=t[:], in_=gw_ext[k0:k0 + KT[k], :])
                    gw.append(t)
                    k0 += KT[k]
                gwT = {}
                q0 = 0
                for q in range(len(co_t)):
                    m0 = 0
                    for m in range(NK):
                        t = pre.tile([co_t[q], KT[m]], f32, name=f"gwT{q}{m}", bufs=1)
                        nc.sync.dma_start(
                            out=t[:], in_=gwT_ext[q0:q0 + co_t[q], m0:m0 + KT[m]])
                        gwT[(q, m)] = t
                        m0 += KT[m]
                    q0 += co_t[q]
                ad_t, as_t = [], []
                q0 = 0
                for q in range(len(co_t)):
                    t1 = pre.tile([co_t[q], H], f32, name=f"ad{q}", bufs=1)
                    nc.sync.dma_start(out=t1[:], in_=ad_ext[q0:q0 + co_t[q], :])
                    t2 = pre.tile([co_t[q], H], f32, name=f"as{q}", bufs=1)
                    nc.sync.dma_start(out=t2[:], in_=as_ext[q0:q0 + co_t[q], :])
                    ad_t.append(t1)
                    as_t.append(t2)
                    q0 += co_t[q]

                # g_d/g_s [cm, H] = gat_w @ A  (lhsT = gwT tiles)
                gd_sb, gs_sb = [], []
                for m in range(NK):
                    for name, amat, dst in (("gd", ad_t, gd_sb), ("gs", as_t, gs_sb)):
                        ps = prep.tile([KT[m], H], f32, space="PSUM",
                                       name="gps", uniquify=True)
                        for q in range(len(co_t)):
                            nc.tensor.matmul(
                                ps[:], lhsT=gwT[(q, m)][:], rhs=amat[q][:],
                                start=(q == 0), stop=(q == len(co_t) - 1))
                        sb = pre.tile([KT[m], H], f32, name=f"{name}sb{m}", bufs=1)
                        nc.scalar.copy(sb[:], ps[:])
                        dst.append(sb)

                # per output k-tile (ci rows of Wfull)
                for m in range(NK):
                    w2ps = prep.tile([KT[m], OUT], f32, space="PSUM", name="w2ps", uniquify=True)
                    for k in range(NK):
                        nc.tensor.matmul(w2ps[:], lhsT=lwT[(k, m)][:], rhs=gw[k][:],
                                         start=(k == 0), stop=(k == NK - 1))
                    wsps = prep.tile([KT[m], H], f32, space="PSUM", name="wsps", uniquify=True)
                    for k in range(NK):
                        nc.tensor.matmul(wsps[:], lhsT=lwT[(k, m)][:], rhs=gs_sb[k][:],
                                         start=(k == 0), stop=(k == NK - 1))
                    wdps = prep.tile([KT[m], H], f32, space="PSUM", name="wdps", uniquify=True)
                    for k in range(NK):
                        nc.tensor.matmul(wdps[:], lhsT=lwT[(k, m)][:], rhs=gd_sb[k][:],
                                         start=(k == 0), stop=(k == NK - 1))
                    sc = 1.0 / L
                    nc.scalar.mul(wfull[m][:, 0:OUT], w2ps[:], sc)
                    nc.scalar.mul(wfull[m][:, OUT:OUT + H], wsps[:], sc)
                    nc.scalar.mul(wfull[m][:, OUT + H:XTW], wdps[:], sc)

            # ---------------- phase 1 ----------------
            with tc.tile_pool(name="p1", bufs=4) as p1, \
                 tc.tile_pool(name="p1p", bufs=2, space="PSUM") as p1p, \
                 tc.tile_pool(name="p1pt", bufs=3, space="PSUM") as p1pt:
                for b in range(NB):
                    x_t = p1.tile([P, L * CIN], f32, name="x_t")
                    nc.sync.dma_start(out=x_t[:], in_=x_ext[b * P:(b + 1) * P, :])
                    xm = p1.tile([P, CIN], f32, name="xm")
                    nc.vector.tensor_reduce(
                        out=xm[:],
                        in_=x_t[:].rearrange("p (l c) -> p c l", l=L),
                        op=mybir.AluOpType.add, axis=mybir.AxisListType.X)
                    xt_ps = p1p.tile([P, XTW], f32, space="PSUM", name="xt_ps")
                    k0 = 0
                    for k in range(NK):
                        tr_ps = p1pt.tile([KT[k], P], f32, space="PSUM", name="tr_ps")
                        nc.tensor.transpose(
                            tr_ps[:], xm[:, k0:k0 + KT[k]], ident[:])
                        xmT = p1.tile([KT[k], P], f32r, name="xmT")
                        nc.scalar.copy(xmT[:], tr_ps[:])
                        nc.tensor.matmul(
                            xt_ps[:], lhsT=xmT[:], rhs=wfull[k][:],
                            start=(k == 0), stop=(k == NK - 1))
                        k0 += KT[k]
                    xts_sb = p1.tile([P, ROWL], bf16, name="xts_sb")
                    nc.scalar.memzero(xts_sb[:, XTW:ROWL])
                    nc.vector.tensor_add(xts_sb[:, 0:XTW], xt_ps[:], bfull_t[:])
                    nc.scalar.dma_start(
                        out=xts_loc[b * P:(b + 1) * P, :], in_=xts_sb[:])
                    sd_sb = p1.tile([P, SROW], bf16, name="sd_sb")
                    nc.scalar.memzero(sd_sb[:, H:SROW])
                    nc.vector.tensor_copy(sd_sb[:, 0:H], xts_sb[:, OUT + H:XTW])
                    nc.scalar.dma_start(
                        out=sdst_loc[b * P:(b + 1) * P, :], in_=sd_sb[:])

            if phases != "p1":
                nc.gpsimd.collective_compute(
                    "AllGather", mybir.AluOpType.bypass,
                    replica_groups=[list(range(c["NCORES"]))],
                    ins=[xts_loc[:]], outs=[xts_all[:]])

        # ---------------- phase 2 ----------------
        with tc.tile_pool(name="p2", bufs=2) as p2, \
             tc.tile_pool(name="p2s", bufs=2) as p2s, \
             tc.tile_pool(name="p2p", bufs=4, space="PSUM") as p2p:
            bias_t = p2s.tile([P, OUT], f32, bufs=1)
            nc.sync.dma_start(out=bias_t[:], in_=bias_ext[:])
            ng_run = {"full": NG, "g1": 1, "g2": 2}.get(phases, 0)
            for g in range(ng_run):
                b0 = g * G
                nb = min(NB - b0, G)
                ilo_t = p2.tile([P, G * CL * 8], i16, name="ilo_t")
                nc.sync.dma_start(out=ilo_t[:, :nb * CL * 8],
                                  in_=ilo_ext[g, :, :nb * CL * 8])
                ihi_t = p2.tile([P, G * CH * 8], i16, name="ihi_t")
                nc.sync.dma_start(out=ihi_t[:, :nb * CH * 8],
                                  in_=ihi_ext[g, :, :nb * CH * 8])
                icol_t = p2.tile([P, G * CT * 8], i16, name="icol_t")
                nc.sync.dma_start(out=icol_t[:, :nb * CT * 8],
                                  in_=icol_ext[g, :, :nb * CT * 8])
                s_t = p2.tile([P, G * CT * P], fp8e4, name="s_t")
                for j in range(nb):
                    nc.sync.dma_start(
                        out=s_t[:, j * CT * P:(j + 1) * CT * P],
                        in_=s_ext[b0 + j, :, :])

                xg_lo = p2.tile([P, G * CL * ROWL], bf16, name="xg_lo", bufs=3)
                nc.gpsimd.dma_gather(
                    out_ap=xg_lo[:, :nb * CL * ROWL].rearrange(
                        "p (c e) -> p c e", e=ROWL),
                    in_ap=xts_all[0:HALF, :], idxs_ap=ilo_t[:, :nb * CL * 8],
                    num_idxs=nb * CL * P, num_idxs_reg=nb * CL * P,
                    elem_size=ROWL, single_packet=False)
                xg_hi = p2.tile([P, G * CH * ROWL], bf16, name="xg_hi", bufs=3)
                nc.gpsimd.dma_gather(
                    out_ap=xg_hi[:, :nb * CH * ROWL].rearrange(
                        "p (c e) -> p c e", e=ROWL),
                    in_ap=xts_all[HALF:NPAD, :], idxs_ap=ihi_t[:, :nb * CH * 8],
                    num_idxs=nb * CH * P, num_idxs_reg=nb * CH * P,
                    elem_size=ROWL, single_packet=False)
                sd_t = p2.tile([P, G * CT * SROW], bf16, name="sd_t", bufs=3)
                nc.gpsimd.dma_gather(
                    out_ap=sd_t[:, :nb * CT * SROW].rearrange(
                        "p (c e) -> p c e", e=SROW),
                    in_ap=sdst_loc[:], idxs_ap=icol_t[:, :nb * CT * 8],
                    num_idxs=nb * CT * P, num_idxs_reg=nb * CT * P,
                    elem_size=SROW, single_packet=False)

                xl3 = xg_lo[:].rearrange("p (c e) -> p c e", e=ROWL)
                xh3 = xg_hi[:].rearrange("p (c e) -> p c e", e=ROWL)
                sd3 = sd_t[:].rearrange("p (c e) -> p c e", e=SROW)
                s3 = s_t[:].rearrange("p (c t) -> p c t", t=P)

                for j in range(nb):
                    b = b0 + j
                    # logits = s_src(gathered) + s_dst(gathered)  [P, CT*4]
                    lg = p2.tile([P, CT * H], f32, name="lg")
                    lg3 = lg[:].rearrange("p (c s) -> p c s", s=H)
                    nc.vector.tensor_add(
                        lg3[:, 0:CL, :],
                        xl3[:, j * CL:(j + 1) * CL, OUT:OUT + H],
                        sd3[:, j * CT:j * CT + CL, 0:H])
                    nc.vector.tensor_add(
                        lg3[:, CL:CT, :],
                        xh3[:, j * CH:(j + 1) * CH, OUT:OUT + H],
                        sd3[:, j * CT + CL:(j + 1) * CT, 0:H])
                    # leaky relu
                    lgm = p2.tile([P, CT * H], f32, name="lgm")
                    nc.vector.tensor_scalar_mul(lgm[:], lg[:], NEG)
                    nc.vector.tensor_tensor(
                        out=lg[:], in0=lgm[:], in1=lg[:], op=mybir.AluOpType.max)
                    # w = exp
                    w_bf = p2.tile([P, CT * H], bf16, name="w_bf")
                    nc.scalar.activation(
                        w_bf[:], lg[:], mybir.ActivationFunctionType.Exp)
                    # pair-duplicate then int32-broadcast to [P, CT*OUT]
                    wpp = p2.tile([P, CT * H * 2], bf16, name="wpp")
                    wpp3 = wpp[:].rearrange("p (k d) -> p k d", d=2)
                    wb3 = w_bf[:].rearrange("p (k o) -> p k o", o=1)
                    nc.vector.tensor_copy(wpp3[:, :, 0:1], wb3)
                    nc.vector.tensor_copy(wpp3[:, :, 1:2], wb3)
                    wr = p2.tile([P, CT * OUT], bf16, name="wr")
                    wr_i3 = wr[:].bitcast(mybir.dt.int32).rearrange(
                        "p (k r) -> p k r", r=D // 2)
                    wpp_i3 = wpp[:].bitcast(mybir.dt.int32).rearrange(
                        "p (k o) -> p k o", o=1)
                    nc.vector.tensor_copy(
                        wr_i3, wpp_i3.to_broadcast([P, CT * H, D // 2]))
                    # rhs = [w * xt | w]
                    rhs = p2.tile([P, CT * (OUT + H)], bf16, name="rhs")
                    rhs3 = rhs[:].rearrange("p (c e) -> p c e", e=OUT + H)
                    wr3 = wr[:].rearrange("p (c e) -> p c e", e=OUT)
                    nc.vector.tensor_mul(
                        rhs3[:, 0:CL, 0:OUT],
                        xl3[:, j * CL:(j + 1) * CL, 0:OUT], wr3[:, 0:CL, :])
                    nc.vector.tensor_mul(
                        rhs3[:, CL:CT, 0:OUT],
                        xh3[:, j * CH:(j + 1) * CH, 0:OUT], wr3[:, CL:CT, :])
                    nc.vector.tensor_copy(
                        rhs3[:, :, OUT:OUT + H],
                        w_bf[:].rearrange("p (c s) -> p c s", s=H))
                    # accumulate [numer | denom] over chunks
                    ps_b = p2p.tile([P, OUT + H], f32, space="PSUM", name="ps_b")
                    for cc in range(CT):
                        nc.tensor.matmul(
                            ps_b[:], lhsT=s3[:, j * CT + cc, :],
                            rhs=rhs3[:, cc, :],
                            start=(cc == 0), stop=(cc == CT - 1))
                    # finalize: out = numer * (1/denom) + bias
                    rd = p2.tile([P, H], f32, name="rd")
                    nc.vector.reciprocal(rd[:], ps_b[:, OUT:OUT + H])
                    rdr = p2.tile([P, OUT], f32, name="rdr")
                    rd3 = rd[:].rearrange("p (h o) -> p h o", o=1)
                    nc.vector.tensor_copy(
                        rdr[:].rearrange("p (h e) -> p h e", e=D),
                        rd3.to_broadcast([P, H, D]))
                    outv = p2.tile([P, OUT], f32, name="outv")
                    nc.vector.tensor_mul(outv[:], ps_b[:, 0:OUT], rdr[:])
                    nc.vector.tensor_add(outv[:], outv[:], bias_t[:])
                    nc.scalar.dma_start(
                        out=out_ext[b * P:(b + 1) * P, :], in_=outv[:])

    nc.finalize()
    return nc


# ---------------------------------------------------------------------------
# entry points
# ---------------------------------------------------------------------------

def run_spmd(nc, in_maps, cfg, trace=False):
    from concourse.bass_utils import run_bass_kernel_spmd

    return run_bass_kernel_spmd(
        nc, in_maps, list(range(cfg["NCORES"])), trace=trace)


def assemble_output(cfg, results):
    out = np.zeros((cfg["N"], cfg["OUT"]), np.float32)
    for cid in range(cfg["NCORES"]):
        n0 = cid * cfg["NP"]
        n1 = min(cfg["N"], n0 + cfg["NP"])
        if n1 > n0:
            out[n0:n1] = results[cid]["out"][0:n1 - n0]
    return out


def run_full(inputs, trace=False):
    cfg = derive_cfg(default_cfg())
    in_maps, CL, CH = prep_inputs(
        cfg, inputs["x"], inputs["edge_index"], inputs["lin_w"],
        inputs["lin_b"], inputs["gat_w"], inputs["att"], inputs["gat_bias"])
    nc = build_nc(cfg, CL, CH)
    r = run_spmd(nc, in_maps, cfg, trace=trace)
    return assemble_output(cfg, r.results), r


def kernel(**inputs):
    out, _ = run_full(inputs, trace=False)
    return out

